# revision 1
# baseline (speedup 1.0000x reference)
"""Distributed single-head attention for TRN2 (8 NeuronCores).

Reference computation (per batch b):
    q = x @ Wq; k = x @ Wk; v = x @ Wv          (x: [S, E])
    s = (q @ k.T) / sqrt(DK) - 1e15 * mask
    out = softmax(s, axis=-1) @ v               ([S, DV])

Sharding: 8 cores = 4 batches x 2 sequence halves. Each core computes
attention for 1024 queries of one batch; K/V are recomputed per core from
the full sequence (softmax is permutation invariant over keys, so the
sequence is rotated per-core to put its queries first).

Host-prepared per-core layout:
  - xt  [E, S]  bf16: x_b^T, sequence permuted, queries first.
  - wq/wk/wv [P, EC*DK] bf16: packed [p, c*DK+d] = W[c*128+p, d]
                (wq pre-scaled by 1/sqrt(DK)).
  - mi  [P, KTILES*SQ] bf16: inverted mask (1-m), keys permuted like xt,
                stored partition-major (mi[p, t*SQ+q] = (1-m)[key t*128+p, q])
                so mask DMAs read fully sequential DRAM.
  - out [DV, SQ] bf16: attention output transposed; host casts + transposes.

Schedule notes (what made this fast; found via perfetto/ntff traces):
  - x is loaded in half-chunks in first-use order (wv, then xa chunks ~1.5
    chunks ahead of xb), so the first projection matmul starts as soon as
    one 256KB piece + its ~2us DMA completion receipt lands.
  - mask DMAs (4 x 1MB) are emitted on the SAME sync HWDGE queue after the
    x loads: per-engine HWDGE FIFO delays mask traffic behind the
    projection-critical x stream for free. (On their own SWDGE queue they
    run concurrently and steal half the HBM bandwidth; SWDGE also measured
    ~3x slower per transfer here.)
  - projections: chunk loop c=0..7 computes VT slabs 0-1 + QT + all 4 KT
    slabs (8 matmuls/chunk, exactly 8 PSUM banks, LDWEIGHTS fully hidden),
    then VT slabs 2-3 on banks freed by the first vt copies.
  - V is transposed [dv,k] -> [k,dv] with TWO grouped xbar DMA transposes
    (3D output AP covering 8 tiles each), not 16 single-tile ones: 16
    transposes exhaust the 8 DMA completion-semaphore lanes and serialize
    later DMAs behind them.
  - attention loop, software-pipelined with lead 3 (PE issues score
    matmuls for tile t+3 before the OT matmuls of tile t, so the PE queue
    always holds independent work while the exp -> mask chain completes).
    Per PAIR of key tiles: two exps (ACT), then ONE wide [128, 2048] DVE
    mask multiply (host-inverted mask, 2x mode) and ONE wide DVE
    accumulate into a pair-shaped running sum. ~1.35us/tile steady state,
    roughly balanced across ACT/DVE/PE.
  - softmax denominator costs zero matmuls in the loop: the accumulated
    masked-P pair-sum is folded by one DVE add + a single ones[128,128]
    stationary matmul pair, which yields rowsum(q) REPLICATED across all
    output partitions. Normalization is then one elementwise multiply
    against the OT PSUM accumulator in its native [dv, q] layout (using
    reciprocal_approx_fast) — no on-device transposes; the host transposes
    the [dv, q] result, which is pure layout.
"""
import math
from contextlib import ExitStack

import ml_dtypes
import numpy as np

import concourse.bass as bass
import concourse.tile as tile
from concourse import bacc, bass_isa, masks, mybir
from concourse.bass_utils import run_bass_kernel_spmd

B, S, E, DK, DV = 4, 2048, 1024, 128, 128
SQ = S // 2  # queries per core
P = 128  # SBUF partitions
EC = E // P  # contraction chunks for projections
KTILES = S // P  # key tiles
QC = SQ // P  # query chunks (epilogue)
LEAD = 3  # score-matmul software-pipeline depth

f32 = mybir.dt.float32
bf16 = mybir.dt.bfloat16

# test.py pokes these to get profiling info
TRACE = False
LAST_RESULT = None


def build():
    nc = bacc.Bacc()
    xt = nc.declare_dram_parameter("xt", [E, S], bf16, isOutput=False)
    wq = nc.declare_dram_parameter("wq", [P, EC * DK], bf16, isOutput=False)
    wk = nc.declare_dram_parameter("wk", [P, EC * DK], bf16, isOutput=False)
    wv = nc.declare_dram_parameter("wv", [P, EC * DV], bf16, isOutput=False)
    mi = nc.declare_dram_parameter("mi", [P, KTILES * SQ], bf16, isOutput=False)
    out = nc.declare_dram_parameter("out", [P, SQ], bf16, isOutput=True)

    with ExitStack() as ctx:
        tc = ctx.enter_context(tile.TileContext(nc))
        const_pool = ctx.enter_context(tc.tile_pool(name="const", bufs=1))
        in_pool = ctx.enter_context(tc.tile_pool(name="inputs", bufs=1))
        proj_sb = ctx.enter_context(tc.tile_pool(name="proj", bufs=1))
        p_pool = ctx.enter_context(tc.tile_pool(name="p", bufs=3))
        stat = ctx.enter_context(tc.tile_pool(name="stat", bufs=1))
        proj_ctx = ctx.enter_context(ExitStack())
        proj_ps = proj_ctx.enter_context(
            tc.tile_pool(name="proj_ps", bufs=8, space="PSUM")
        )

        # running sum over PAIRS of masked P tiles (pair-shaped so each
        # accumulate is one wide DVE op; the final ones-matmuls fold both
        # halves into rs_rep via PSUM accumulation)
        acc_d = stat.tile([P, 2 * SQ], bf16)
        ones_mat = const_pool.tile([P, P], bf16)
        nc.gpsimd.memset(ones_mat[:], 1.0)
        # preload the exp table set off the critical path
        warm = const_pool.tile([1, 2], f32)
        nc.gpsimd.memset(warm[:], 0.0)
        nc.scalar.activation(warm[:], warm[:], mybir.ActivationFunctionType.Exp)

        # --- input loads: sync HWDGE queue, first-use order, x split in
        # half-chunks so the first arrival (and its ~2us completion receipt)
        # comes early. xa = query-half columns, xb = second key half.
        w_sb = {}
        for name in ("wq", "wk", "wv"):
            w_sb[name] = in_pool.tile([P, EC * DK], bf16, tag=name, name=f"w_{name}")
        xa_sb = []
        xb_sb = []
        for c in range(EC):
            xa_sb.append(in_pool.tile([P, SQ], bf16, tag=f"xa{c}", name=f"xa{c}"))
            xb_sb.append(in_pool.tile([P, SQ], bf16, tag=f"xb{c}", name=f"xb{c}"))

        # xa chunks feed 6 of each chunk's 8 matmuls; keep them ~1.5
        # chunks ahead of xb in the issue stream so the chunk loop never
        # waits (each issue costs ~650ns of sync-sequencer time).
        nc.sync.dma_start(w_sb["wv"][:], wv[:, :])
        nc.sync.dma_start(xa_sb[0][:], xt[0:P, 0:SQ])
        nc.sync.dma_start(w_sb["wq"][:], wq[:, :])
        nc.sync.dma_start(w_sb["wk"][:], wk[:, :])
        nc.sync.dma_start(xa_sb[1][:], xt[P : 2 * P, 0:SQ])
        nc.sync.dma_start(xa_sb[2][:], xt[2 * P : 3 * P, 0:SQ])
        nc.sync.dma_start(xb_sb[0][:], xt[0:P, SQ:S])
        for c in range(3, EC):
            nc.sync.dma_start(xa_sb[c][:], xt[c * P : (c + 1) * P, 0:SQ])
            nc.sync.dma_start(
                xb_sb[c - 2][:], xt[(c - 2) * P : (c - 1) * P, SQ:S]
            )
        nc.sync.dma_start(xb_sb[EC - 2][:], xt[(EC - 2) * P : (EC - 1) * P, SQ:S])
        nc.sync.dma_start(xb_sb[EC - 1][:], xt[(EC - 1) * P : EC * P, SQ:S])

        # masks: 4 grouped quarter-DMAs on the SAME sync HWDGE queue,
        # emitted after the x loads: per-engine HWDGE FIFO order delays the
        # mask traffic behind the projection-critical x stream for free.
        # The host stores the mask partition-major ([p, t*SQ+q] =
        # mask^T[t*128+p, q]) so each transfer reads fully sequential DRAM
        # (the on-the-fly rearrange was descriptor-scattered and ran slow).
        m_all = []
        for qtr in range(4):
            mq = in_pool.tile([P, 4 * SQ], bf16, tag=f"mq{qtr}", name=f"mq{qtr}")
            m_all.append(mq)
            nc.sync.dma_start(
                mq[:], mi[:, qtr * 4 * SQ : (qtr + 1) * 4 * SQ]
            )

        def m_tile(t):
            return m_all[t // 4][:, (t % 4) * SQ : (t % 4 + 1) * SQ]

        # --- projections ---
        qt_sb = proj_sb.tile([P, SQ], bf16)
        kt_sb = [
            proj_sb.tile([P, 512], bf16, tag=f"kt{s}", name=f"kt{s}")
            for s in range(4)
        ]
        # vt in two halves of [d, 1024]; each is group-transposed to 8 v
        # tiles laid out [k, t*128+dv]
        vt_sb = [
            proj_sb.tile([P, SQ], bf16, tag=f"vt{h}", name=f"vt{h}")
            for h in range(2)
        ]
        v_sb = [
            proj_sb.tile([P, 8 * DV], bf16, tag=f"v{h}", name=f"v{h}")
            for h in range(2)
        ]

        def v_tile(t):
            return v_sb[t // 8][:, (t % 8) * DV : (t % 8 + 1) * DV]

        # PSUM: one pool, 8 banks, rotating in allocation order.
        # Phase A (chunk loop): vtA0 vtA1 qt0 qt1 kt0..kt3 = 8 banks.
        # Phase B: vtB0 vtB1 reuse vtA0/vtA1 banks (WAR on their copies).
        ps = {}
        for tag in ("vtA0", "vtA1", "qt0", "qt1", "kt0", "kt1", "kt2", "kt3"):
            ps[tag] = proj_ps.tile([P, 512], f32, tag="pps", name=f"ps_{tag}")

        def wslice(w, c):
            return w[:, c * DK : (c + 1) * DK]

        def xs(c, j):  # x chunk c, 512-column slab j (0..3)
            half = xa_sb if j < 2 else xb_sb
            jj = j % 2
            return half[c][:, jj * 512 : (jj + 1) * 512]

        for c in range(EC):
            st_flags = dict(start=(c == 0), stop=(c == EC - 1))
            for j in range(2):  # VT slabs 0-1 (keys 0..1023)
                nc.tensor.matmul(
                    ps[f"vtA{j}"][:], wslice(w_sb["wv"], c), xs(c, j), **st_flags
                )
            for j in range(2):  # QT (queries = columns 0..1023)
                nc.tensor.matmul(
                    ps[f"qt{j}"][:], wslice(w_sb["wq"], c), xs(c, j), **st_flags
                )
            for j in range(4):  # KT all 4 slabs
                nc.tensor.matmul(
                    ps[f"kt{j}"][:], wslice(w_sb["wk"], c), xs(c, j), **st_flags
                )

        # PSUM -> SBUF copies, split across DVE and ACT so neither is the
        # critical path: DVE feeds the V transpose, ACT feeds the first
        # score matmuls (qt, kt0).
        nc.vector.tensor_copy(vt_sb[0][:, 0:512], ps["vtA0"][:])
        nc.scalar.copy(qt_sb[:, 0:512], ps["qt0"][:])
        nc.vector.tensor_copy(vt_sb[0][:, 512:1024], ps["vtA1"][:])
        nc.scalar.copy(qt_sb[:, 512:1024], ps["qt1"][:])
        nc.vector.tensor_copy(kt_sb[1][:], ps["kt1"][:])
        nc.scalar.copy(kt_sb[0][:], ps["kt0"][:])
        nc.vector.tensor_copy(kt_sb[3][:], ps["kt3"][:])
        nc.scalar.copy(kt_sb[2][:], ps["kt2"][:])

        # V transposes: TWO grouped xbar ops; out[k, t, dv] = vt[dv, t*128+k]
        nc.sync.dma_start_transpose(
            v_sb[0][:].rearrange("p (t d) -> p t d", t=8), vt_sb[0][:]
        )

        # VT slabs 2-3 (phase B) on the banks freed by the vtA copies
        ps["vtB0"] = proj_ps.tile([P, 512], f32, tag="pps", name="ps_vtB0")
        ps["vtB1"] = proj_ps.tile([P, 512], f32, tag="pps", name="ps_vtB1")
        for c in range(EC):
            st_flags = dict(start=(c == 0), stop=(c == EC - 1))
            for j in range(2):
                nc.tensor.matmul(
                    ps[f"vtB{j}"][:],
                    wslice(w_sb["wv"], c),
                    xs(c, 2 + j),
                    **st_flags,
                )
        nc.vector.tensor_copy(vt_sb[1][:, 0:512], ps["vtB0"][:])
        nc.vector.tensor_copy(vt_sb[1][:, 512:1024], ps["vtB1"][:])
        nc.sync.dma_start_transpose(
            v_sb[1][:].rearrange("p (t d) -> p t d", t=8), vt_sb[1][:]
        )

        proj_ctx.close()  # free projection PSUM banks for the loop

        ot_ps = ctx.enter_context(tc.tile_pool(name="ot_ps", bufs=1, space="PSUM"))
        st_ctx = ctx.enter_context(ExitStack())
        st_ps = st_ctx.enter_context(
            tc.tile_pool(name="st_ps", bufs=LEAD, space="PSUM")
        )

        ot = ot_ps.tile([P, SQ], f32)  # OT [dv, q] accumulator

        st_tiles = []

        def score_mms(t):
            st = st_ps.tile([P, SQ], f32, tag="st", name=f"st{t}")
            st_tiles.append(st)
            s, o = divmod(t, 4)
            for j in range(2):
                nc.tensor.matmul(
                    st[:, j * 512 : (j + 1) * 512],
                    kt_sb[s][:, o * P : (o + 1) * P],
                    qt_sb[:, j * 512 : (j + 1) * 512],
                    start=True,
                    stop=True,
                )

        for t in range(LEAD):
            score_mms(t)

        def m_pair(r):  # mask columns for tiles 2r, 2r+1 (contiguous)
            qtr, off = divmod(r, 2)
            return m_all[qtr][:, off * 2 * SQ : (off + 1) * 2 * SQ]

        for r in range(KTILES // 2):
            pp = p_pool.tile([P, 2 * SQ], bf16, tag="p", name=f"pp{r}")
            for h in range(2):
                t = 2 * r + h
                nc.scalar.activation(
                    pp[:, h * SQ : (h + 1) * SQ],
                    st_tiles[t][:],
                    mybir.ActivationFunctionType.Exp,
                )
            # zero masked entries for both tiles in one wide op
            nc.vector.tensor_mul(pp[:], pp[:], m_pair(r))
            # accumulate the pair into the pair-shaped denominator sum
            if r == 0:
                nc.vector.tensor_copy(acc_d[:], pp[:])
            else:
                nc.vector.tensor_add(acc_d[:], acc_d[:], pp[:])
            for h in range(2):
                t = 2 * r + h
                if t + LEAD < KTILES:
                    score_mms(t + LEAD)
                for j in range(2):
                    nc.tensor.matmul(
                        ot[:, j * 512 : (j + 1) * 512],
                        v_tile(t),
                        pp[:, h * SQ + j * 512 : h * SQ + (j + 1) * 512],
                        start=(t == 0),
                        stop=(t == KTILES - 1),
                    )

        # --- epilogue: normalize in [dv, q] layout ---
        # ones[128,128]^T @ acc gives rowsum(q) REPLICATED on every output
        # partition in one matmul pair; normalization is then a plain
        # elementwise multiply against the PSUM accumulator — no transposes
        # on device (the host transposes the [dv, q] result).
        st_ctx.close()
        rsrep_pool = ctx.enter_context(
            tc.tile_pool(name="rsrep_ps", bufs=1, space="PSUM")
        )
        # fold the two pair-halves with one DVE add, then one matmul pair
        acc_f = stat.tile([P, SQ], bf16)
        nc.vector.tensor_add(acc_f[:], acc_d[:, 0:SQ], acc_d[:, SQ : 2 * SQ])
        rs_rep = rsrep_pool.tile([P, SQ], f32)
        for j in range(2):
            nc.tensor.matmul(
                rs_rep[:, j * 512 : (j + 1) * 512],
                ones_mat[:],
                acc_f[:, j * 512 : (j + 1) * 512],
                start=True,
                stop=True,
            )
        rcp_rep = stat.tile([P, SQ], f32)
        o_sb = stat.tile([P, SQ], bf16)
        for j in range(2):  # per-half pipeline: recip -> scale -> store
            sl = slice(j * 512, (j + 1) * 512)
            nc.vector.reciprocal_approx_fast(rcp_rep[:, sl], rs_rep[:, sl])
            nc.vector.tensor_mul(o_sb[:, sl], ot[:, sl], rcp_rep[:, sl])
            nc.sync.dma_start(out[:, sl], o_sb[:, sl])

    nc.compile()
    return nc


_NC_CACHE = None


def kernel(inputs, mask, Wq, Wk, Wv):
    global _NC_CACHE, LAST_RESULT
    inputs = np.asarray(inputs)
    mask = np.asarray(mask)
    bf = ml_dtypes.bfloat16
    scale = np.float32(1.0 / math.sqrt(DK))

    def pack_w(w):  # [E, DK] -> [p, c*DK+d] = w[c*128+p, d]
        w = np.asarray(w).astype(bf)
        return np.ascontiguousarray(
            w.reshape(EC, P, DK).transpose(1, 0, 2).reshape(P, EC * DK)
        )

    wq_h = pack_w(np.asarray(Wq) * scale)
    wk_h = pack_w(Wk)
    wv_h = pack_w(Wv)

    if _NC_CACHE is None:
        _NC_CACHE = build()
    nc = _NC_CACHE

    minv = (1 - mask).astype(bf)  # [B, S, S], entries in {0, 1}
    in_maps = []
    for core in range(8):
        b, h = divmod(core, 2)
        q0 = h * SQ
        idx = np.r_[q0:S, 0:q0]  # rotate so this core's queries come first
        xb = inputs[b]  # [S, E] f32
        xt_core = np.ascontiguousarray(xb[idx].T).astype(bf)  # [E, S]
        mt = minv[b, q0 : q0 + SQ, :][:, idx].T  # [S, SQ] keys-major
        # partition-major swizzle: mi_core[p, t*SQ+q] = mt[t*128+p, q]
        mi_core = np.ascontiguousarray(
            mt.reshape(KTILES, P, SQ).transpose(1, 0, 2).reshape(P, KTILES * SQ)
        )
        in_maps.append(
            {"xt": xt_core, "wq": wq_h, "wk": wk_h, "wv": wv_h, "mi": mi_core}
        )

    res = run_bass_kernel_spmd(nc, in_maps, list(range(8)), trace=TRACE)
    LAST_RESULT = res
    outp = np.empty((B, S, DV), np.float32)
    for core in range(8):
        b, h = divmod(core, 2)
        q0 = h * SQ
        o = np.asarray(res.results[core]["out"]).astype(np.float32)  # [DV, SQ]
        outp[b, q0 : q0 + SQ, :] = o.T
    return outp



# revision 7
# speedup vs baseline: 1.2167x; 1.2167x over previous
"""Distributed single-head attention for TRN2 (8 NeuronCores).

Reference computation (per batch b):
    q = x @ Wq; k = x @ Wk; v = x @ Wv          (x: [S, E])
    s = (q @ k.T) / sqrt(DK) - 1e15 * mask
    out = softmax(s, axis=-1) @ v               ([S, DV])

Sharding: 8 cores = 4 batches x 2 sequence halves. Each core computes
attention for 1024 queries of one batch; K/V are recomputed per core from
the full sequence (softmax is permutation invariant over keys, so the
sequence is rotated per-core to put its queries first).

Host-prepared per-core layout:
  - xt  [E, S]  bf16: x_b^T, sequence permuted, queries first.
  - wq/wk/wv [P, EC*DK] bf16: packed [p, c*DK+d] = W[c*128+p, d]
                (wq pre-scaled by 1/sqrt(DK)).
  - mi  [P, KTILES*SQ] bf16: inverted mask (1-m), keys permuted like xt,
                stored partition-major (mi[p, t*SQ+q] = (1-m)[key t*128+p, q])
                so mask DMAs read fully sequential DRAM.
  - out [DV, SQ] bf16: attention output transposed; host casts + transposes.

Schedule (v2 — overlapped):
  - PE warmup: ~28 dummy matmuls on ones starting right after the preamble
    trip the HAM activity monitor, so the PE clock is at 2.4 GHz (not the
    cold 1.2 GHz) by the time the first x chunk lands.
  - x is loaded in 8 FULL-chunk DMAs [128, 2048] (fewer sync-sequencer
    issues than half-chunks); masks follow on the SAME sync HWDGE queue so
    the per-engine FIFO delays mask traffic behind the projection-critical
    x stream for free.
  - chunk loop computes QT + all 4 KT slabs + VT slabs A0/A1 (8 matmuls,
    8 PSUM banks: 6 projA + 2 vt pool). QT/KT copies are staged: qt + kt0
    right away (ACT/DVE), kt1..3 interleaved into the loop's ACT stream —
    so the attention loop starts ~2us after the last chunk matmul.
  - VT slabs B0/B1 (keys 1024..2047) are computed INSIDE the attention
    loop (8 matmuls appended to pairs 0 and 1), copied + DMA-transposed
    during pairs 2-3 — PE has idle capacity there since the loop is
    ACT/DVE bound. PSUM: st(4) + ot(2) + vt(2) = 8 banks exactly.
  - attention loop over PAIRS of key tiles, score lead = 2 tiles: two exps
    (ACT), one wide [128, 2048] DVE mask multiply (host-inverted mask,
    2x mode) and one wide DVE accumulate into a pair-shaped running sum.
  - softmax denominator costs zero matmuls in the loop: the accumulated
    masked-P pair-sum is folded by one DVE add + a single ones[128,128]
    stationary matmul pair, which yields rowsum(q) REPLICATED across all
    output partitions. Normalization is then one elementwise multiply
    against the OT PSUM accumulator in its native [dv, q] layout (using
    reciprocal_approx_fast) — no on-device transposes; the host transposes
    the [dv, q] result, which is pure layout.
"""
import math
from contextlib import ExitStack

import ml_dtypes
import numpy as np

import concourse.bass as bass
import concourse.tile as tile
from concourse import bacc, bass_isa, masks, mybir
from concourse.bass_utils import run_bass_kernel_spmd

B, S, E, DK, DV = 4, 2048, 1024, 128, 128
SQ = S // 2  # queries per core
P = 128  # SBUF partitions
EC = E // P  # contraction chunks for projections
KTILES = S // P  # key tiles
QC = SQ // P  # query chunks (epilogue)
N_WARM = 28  # PE warmup matmuls (HAM clock-gate release)

f32 = mybir.dt.float32
bf16 = mybir.dt.bfloat16

# test.py pokes these to get profiling info
TRACE = False
LAST_RESULT = None


def build():
    nc = bacc.Bacc()
    xt = nc.declare_dram_parameter("xt", [E, S], bf16, isOutput=False)
    wq = nc.declare_dram_parameter("wq", [P, EC * DK], bf16, isOutput=False)
    wk = nc.declare_dram_parameter("wk", [P, EC * DK], bf16, isOutput=False)
    wv = nc.declare_dram_parameter("wv", [P, EC * DV], bf16, isOutput=False)
    mi = nc.declare_dram_parameter("mi", [P, KTILES * SQ], bf16, isOutput=False)
    out = nc.declare_dram_parameter("out", [P, SQ], bf16, isOutput=True)

    with ExitStack() as ctx:
        tc = ctx.enter_context(tile.TileContext(nc))
        const_pool = ctx.enter_context(tc.tile_pool(name="const", bufs=1))
        in_pool = ctx.enter_context(tc.tile_pool(name="inputs", bufs=1))
        proj_sb = ctx.enter_context(tc.tile_pool(name="proj", bufs=1))
        p_pool = ctx.enter_context(tc.tile_pool(name="p", bufs=3))
        stat = ctx.enter_context(tc.tile_pool(name="stat", bufs=1))

        # vt PSUM pool outlives the projection pool: slabs A0/A1 during the
        # chunk loop, B0/B1 (reusing the same 2 banks) inside the loop.
        vt_ps = ctx.enter_context(tc.tile_pool(name="vt_ps", bufs=2, space="PSUM"))

        ones_mat = const_pool.tile([P, P], bf16)
        nc.gpsimd.memset(ones_mat[:], 1.0)
        # preload the exp table set off the critical path
        warm = const_pool.tile([1, 2], f32)
        nc.gpsimd.memset(warm[:], 0.0)
        nc.scalar.activation(warm[:], warm[:], mybir.ActivationFunctionType.Exp)

        # --- PE warmup: release the HAM clock gate before real work ---
        with tc.tile_pool(name="warm_ps", bufs=1, space="PSUM") as warm_pool:
            wps = warm_pool.tile([P, P], f32)
            for i in range(N_WARM):
                nc.tensor.matmul(
                    wps[:], ones_mat[:], ones_mat[:],
                    start=(i == 0), stop=(i == N_WARM - 1),
                )

        # running sum over PAIRS of masked P tiles (pair-shaped so each
        # accumulate is one wide DVE op; the final ones-matmuls fold both
        # halves into rs_rep via PSUM accumulation)
        acc_d = stat.tile([P, 2 * SQ], bf16)

        # --- input loads: sync HWDGE queue, first-use order. Full-chunk
        # x DMAs (512KB each): chunk c = xt rows [c*128, (c+1)*128), all
        # 2048 columns (queries first, then the second key half).
        w_sb = {}
        for name in ("wq", "wk", "wv"):
            w_sb[name] = in_pool.tile([P, EC * DK], bf16, tag=name, name=f"w_{name}")
        x_sb = []
        for c in range(EC):
            x_sb.append(in_pool.tile([P, S], bf16, tag=f"x{c}", name=f"x{c}"))

        nc.sync.dma_start(w_sb["wq"][:], wq[:, :])
        nc.sync.dma_start(x_sb[0][:], xt[0:P, :])
        nc.sync.dma_start(w_sb["wk"][:], wk[:, :])
        nc.sync.dma_start(w_sb["wv"][:], wv[:, :])
        for c in range(1, EC):
            nc.sync.dma_start(x_sb[c][:], xt[c * P : (c + 1) * P, :])

        # masks: 4 grouped quarter-DMAs on the SAME sync HWDGE queue,
        # emitted after the x loads: per-engine HWDGE FIFO order delays the
        # mask traffic behind the projection-critical x stream for free.
        m_all = []
        for qtr in range(4):
            mq = in_pool.tile([P, 4 * SQ], bf16, tag=f"mq{qtr}", name=f"mq{qtr}")
            m_all.append(mq)
            nc.sync.dma_start(mq[:], mi[:, qtr * 4 * SQ : (qtr + 1) * 4 * SQ])

        def m_pair(r):  # mask columns for tiles 2r, 2r+1 (contiguous)
            qtr, off = divmod(r, 2)
            return m_all[qtr][:, off * 2 * SQ : (off + 1) * 2 * SQ]

        # --- projections ---
        qt_sb = proj_sb.tile([P, SQ], bf16)
        kt_sb = [
            proj_sb.tile([P, 512], bf16, tag=f"kt{s}", name=f"kt{s}")
            for s in range(4)
        ]
        # vt in two halves of [d, 1024]; each is group-transposed to v
        # tiles laid out [k, t*128+dv]
        vt_sb = [
            proj_sb.tile([P, SQ], bf16, tag=f"vt{h}", name=f"vt{h}")
            for h in range(2)
        ]
        v_sb = [
            proj_sb.tile([P, 8 * DV], bf16, tag=f"v{h}", name=f"v{h}")
            for h in range(2)
        ]

        def v_tile(t):
            return v_sb[t // 8][:, (t % 8) * DV : (t % 8 + 1) * DV]

        def wslice(w, c):
            return w[:, c * DK : (c + 1) * DK]

        def xs(c, j):  # x chunk c, 512-column slab j (0..3)
            return x_sb[c][:, j * 512 : (j + 1) * 512]

        # vt PSUM tiles (2 banks, live through the loop)
        ps_vt = {}
        for tag in ("vtA0", "vtA1"):
            ps_vt[tag] = vt_ps.tile([P, 512], f32, tag="vtps", name=f"ps_{tag}")

        proj_ctx = ExitStack()
        proj_ps = proj_ctx.enter_context(
            tc.tile_pool(name="proj_ps", bufs=6, space="PSUM")
        )
        ps = {}
        for tag in ("qt0", "qt1", "kt0", "kt1", "kt2", "kt3"):
            ps[tag] = proj_ps.tile([P, 512], f32, tag="pps", name=f"ps_{tag}")

        # chunk loop: qt first, then kt, then vtA — so the last chunk
        # finishes the loop-critical qt/kt slabs as early as possible.
        for c in range(EC):
            st_flags = dict(start=(c == 0), stop=(c == EC - 1))
            for j in range(2):  # QT (queries = columns 0..1023)
                nc.tensor.matmul(
                    ps[f"qt{j}"][:], wslice(w_sb["wq"], c), xs(c, j), **st_flags
                )
            for j in range(4):  # KT all 4 slabs
                nc.tensor.matmul(
                    ps[f"kt{j}"][:], wslice(w_sb["wk"], c), xs(c, j), **st_flags
                )
            for j in range(2):  # VT slabs A0-A1 (keys 0..1023)
                nc.tensor.matmul(
                    ps_vt[f"vtA{j}"][:], wslice(w_sb["wv"], c), xs(c, j), **st_flags
                )

        # PSUM -> SBUF copies, ordered so the loop-critical pieces come
        # first on each engine: ACT gets qt (scores need it immediately)
        # then kt3 (last used); DVE gets kt0 first, then the laggards.
        # Neither engine's copy tail delays the loop: the first mask
        # multiply (DVE) waits on the pair-0 exps (ACT) anyway.
        nc.scalar.copy(qt_sb[:, 0:512], ps["qt0"][:])
        nc.vector.tensor_copy(kt_sb[0][:], ps["kt0"][:])
        nc.scalar.copy(qt_sb[:, 512:1024], ps["qt1"][:])
        nc.vector.tensor_copy(kt_sb[1][:], ps["kt1"][:])
        nc.scalar.copy(kt_sb[3][:], ps["kt3"][:])
        nc.vector.tensor_copy(kt_sb[2][:], ps["kt2"][:])
        nc.vector.tensor_copy(vt_sb[0][:, 0:512], ps_vt["vtA0"][:])
        nc.vector.tensor_copy(vt_sb[0][:, 512:1024], ps_vt["vtA1"][:])

        # V transpose A: ONE grouped xbar op; out[k, t, dv] = vt[dv, t*128+k]
        nc.sync.dma_start_transpose(
            v_sb[0][:].rearrange("p (t d) -> p t d", t=8), vt_sb[0][:]
        )

        proj_ctx.close()  # free the 6 qt/kt banks for the loop pools

        # VT slabs B0/B1 reuse the vt banks (WAR on the vtA copies); their
        # matmuls are emitted inside loop pairs 0-1.
        ps_vt["vtB0"] = vt_ps.tile([P, 512], f32, tag="vtps", name="ps_vtB0")
        ps_vt["vtB1"] = vt_ps.tile([P, 512], f32, tag="vtps", name="ps_vtB1")

        def vtb_mms(half):  # half 0 -> chunks 0..3, half 1 -> chunks 4..7
            for c in range(4 * half, 4 * half + 4):
                for j in range(2):
                    nc.tensor.matmul(
                        ps_vt[f"vtB{j}"][:],
                        wslice(w_sb["wv"], c),
                        xs(c, 2 + j),
                        start=(c == 0),
                        stop=(c == EC - 1),
                    )

        ot_ps = ctx.enter_context(tc.tile_pool(name="ot_ps", bufs=1, space="PSUM"))
        st_ctx = ctx.enter_context(ExitStack())
        st_ps = st_ctx.enter_context(
            tc.tile_pool(name="st_ps", bufs=2, space="PSUM")
        )

        ot = ot_ps.tile([P, SQ], f32)  # OT [dv, q] accumulator

        st_tiles = []

        def score_mms(t):
            st = st_ps.tile([P, SQ], f32, tag="st", name=f"st{t}")
            st_tiles.append(st)
            s, o = divmod(t, 4)
            for j in range(2):
                nc.tensor.matmul(
                    st[:, j * 512 : (j + 1) * 512],
                    kt_sb[s][:, o * P : (o + 1) * P],
                    qt_sb[:, j * 512 : (j + 1) * 512],
                    start=True,
                    stop=True,
                )

        score_mms(0)
        score_mms(1)

        for r in range(KTILES // 2):
            pp = p_pool.tile([P, 2 * SQ], bf16, tag="p", name=f"pp{r}")
            for h in range(2):
                t = 2 * r + h
                nc.scalar.activation(
                    pp[:, h * SQ : (h + 1) * SQ],
                    st_tiles[t][:],
                    mybir.ActivationFunctionType.Exp,
                )
            # zero masked entries for both tiles in one wide op
            nc.vector.tensor_mul(pp[:], pp[:], m_pair(r))
            # accumulate the pair into the pair-shaped denominator sum
            if r == 0:
                nc.vector.tensor_copy(acc_d[:], pp[:])
            else:
                nc.vector.tensor_add(acc_d[:], acc_d[:], pp[:])
            for h in range(2):
                t = 2 * r + h
                if t + 2 < KTILES:
                    score_mms(t + 2)
                for j in range(2):
                    nc.tensor.matmul(
                        ot[:, j * 512 : (j + 1) * 512],
                        v_tile(t),
                        pp[:, h * SQ + j * 512 : h * SQ + (j + 1) * 512],
                        start=(t == 0),
                        stop=(t == KTILES - 1),
                    )
            if r < 2:  # VT slabs B0/B1 ride the loop's idle PE capacity
                vtb_mms(r)
            if r == 1:  # B0/B1 copies + per-slab transposes, DVE stream
                nc.vector.tensor_copy(vt_sb[1][:, 0:512], ps_vt["vtB0"][:])
                nc.sync.dma_start_transpose(
                    v_sb[1][:, 0:512].rearrange("p (t d) -> p t d", t=4),
                    vt_sb[1][:, 0:512],
                )
            if r == 2:
                nc.vector.tensor_copy(vt_sb[1][:, 512:1024], ps_vt["vtB1"][:])
                nc.sync.dma_start_transpose(
                    v_sb[1][:, 512:1024].rearrange("p (t d) -> p t d", t=4),
                    vt_sb[1][:, 512:1024],
                )

        # --- epilogue: normalize in [dv, q] layout ---
        # ones[128,128]^T @ acc gives rowsum(q) REPLICATED on every output
        # partition in one matmul pair; normalization is then a plain
        # elementwise multiply against the PSUM accumulator — no transposes
        # on device (the host transposes the [dv, q] result).
        st_ctx.close()
        rsrep_pool = ctx.enter_context(
            tc.tile_pool(name="rsrep_ps", bufs=1, space="PSUM")
        )
        # fold the two pair-halves with one DVE add, then one matmul pair
        acc_f = stat.tile([P, SQ], bf16)
        nc.vector.tensor_add(acc_f[:], acc_d[:, 0:SQ], acc_d[:, SQ : 2 * SQ])
        rs_rep = rsrep_pool.tile([P, SQ], f32)
        for j in range(2):
            nc.tensor.matmul(
                rs_rep[:, j * 512 : (j + 1) * 512],
                ones_mat[:],
                acc_f[:, j * 512 : (j + 1) * 512],
                start=True,
                stop=True,
            )
        rcp_rep = stat.tile([P, SQ], f32)
        o_sb = stat.tile([P, SQ], bf16)
        for j in range(2):  # per-half pipeline: recip -> scale -> store
            sl = slice(j * 512, (j + 1) * 512)
            nc.vector.reciprocal_approx_fast(rcp_rep[:, sl], rs_rep[:, sl])
            nc.vector.tensor_mul(o_sb[:, sl], ot[:, sl], rcp_rep[:, sl])
            nc.sync.dma_start(out[:, sl], o_sb[:, sl])

    nc.compile()
    return nc


_NC_CACHE = None


def kernel(inputs, mask, Wq, Wk, Wv):
    global _NC_CACHE, LAST_RESULT
    inputs = np.asarray(inputs)
    mask = np.asarray(mask)
    bf = ml_dtypes.bfloat16
    scale = np.float32(1.0 / math.sqrt(DK))

    def pack_w(w):  # [E, DK] -> [p, c*DK+d] = w[c*128+p, d]
        w = np.asarray(w).astype(bf)
        return np.ascontiguousarray(
            w.reshape(EC, P, DK).transpose(1, 0, 2).reshape(P, EC * DK)
        )

    wq_h = pack_w(np.asarray(Wq) * scale)
    wk_h = pack_w(Wk)
    wv_h = pack_w(Wv)

    if _NC_CACHE is None:
        _NC_CACHE = build()
    nc = _NC_CACHE

    minv = (1 - mask).astype(bf)  # [B, S, S], entries in {0, 1}
    in_maps = []
    for core in range(8):
        b, h = divmod(core, 2)
        q0 = h * SQ
        idx = np.r_[q0:S, 0:q0]  # rotate so this core's queries come first
        xb = inputs[b]  # [S, E] f32
        xt_core = np.ascontiguousarray(xb[idx].T).astype(bf)  # [E, S]
        mt = minv[b, q0 : q0 + SQ, :][:, idx].T  # [S, SQ] keys-major
        # partition-major swizzle: mi_core[p, t*SQ+q] = mt[t*128+p, q]
        mi_core = np.ascontiguousarray(
            mt.reshape(KTILES, P, SQ).transpose(1, 0, 2).reshape(P, KTILES * SQ)
        )
        in_maps.append(
            {"xt": xt_core, "wq": wq_h, "wk": wk_h, "wv": wv_h, "mi": mi_core}
        )

    res = run_bass_kernel_spmd(nc, in_maps, list(range(8)), trace=TRACE)
    LAST_RESULT = res
    outp = np.empty((B, S, DV), np.float32)
    for core in range(8):
        b, h = divmod(core, 2)
        q0 = h * SQ
        o = np.asarray(res.results[core]["out"]).astype(np.float32)  # [DV, SQ]
        outp[b, q0 : q0 + SQ, :] = o.T
    return outp
